# revision 7
# baseline (speedup 1.0000x reference)
"""Trainium2 Bass kernel for DDN depth-focal loss (nn_DDNLoss) — v3.

Data-parallel over batch B=8 across 8 NeuronCores (1 image per core).
Each core computes sum_pixels(weight * focal(depth_logits, target)); host
sums the 8 partials and divides by B*H*W.

v3 design (vs v2 at ~61us):
  - 84-channel layout (81 + 3 pad) in pixel-major [128, j, c] form: less
    ACT exp work and less DMA than v2's 88.
  - Column pixel mapping (partition = image column mod 128) so the
    raster matmuls write [128, 240] PSUM directly (stationary = width
    masks, moving = height masks) — no DRAM reshape bounce.
  - 8 geometrically-sized strips so the first exp starts as soon as a
    small strip lands and the pipeline tail is short.
  - Winner gather via per-partition Abel summation: each partition's
    pixels live in ~2.5 image columns that only <=18 depth-sorted box
    runs (merged by equal target bin) can cover, so
    x_t = sum_k dg[k] * [t >= 2^rlo_k] with host-differenced planes
    that telescope to the winner's logit. 19 scalar_tensor_tensor ops
    with per-partition PTR thresholds replace v2's 33 global slots.
  - Channel-sum pipeline split across engines: DVE fold1 (84->42),
    Pool engine fold2 (42->21), pool_avg (21->1, the 1/21 absorbed into
    the log constant). Some gather slots also run on Pool.
  - Final partition reduce via a ones-matmul on the idle PE.
"""

import numpy as np
import ml_dtypes

import concourse.bacc as bacc
import concourse.bass as bass
import concourse.mybir as mybir
from concourse import ap_utils, bass_isa, tile
from concourse.bass_utils import run_bass_kernel_spmd

# Problem constants (hardcoded per harness contract).
B, C, H, W, N = 8, 81, 96, 320, 32
P = 128
HW = H * W              # 30720
J = HW // P             # 240 pixel columns per partition
CPS = 84                # sum-region channels padded (81 -> 84 = 2*2*21)
KCAP = 20               # bg + up to 19 merged-bin runs per partition
WE = W + 256            # extended width-iota (two zero-padded 128-chunks)

STRIPJ = [12, 24, 36, 42, 42, 42, 30, 12]
NSTRIP = len(STRIPJ)
JOFF = [sum(STRIPJ[:i]) for i in range(NSTRIP + 1)]
assert JOFF[-1] == J

ALPHA = 0.25
FG_W, BG_W = 13.0, 1.0
DEPTH_MIN, DEPTH_MAX, NUM_BINS = 0.001, 60.0, 80
BIN_SIZE = 2.0 * (DEPTH_MAX - DEPTH_MIN) / (NUM_BINS * (1 + NUM_BINS))
PAD_LOGIT = -20.0
LN2 = float(np.log(2.0))
SIG = 0.0573
CMP_PAD = float(2.0 ** 40)
IOTA_DEAD = 100000.0

F32 = mybir.dt.float32
BF16 = mybir.dt.bfloat16
FP8 = mybir.dt.float8e4
I32 = mybir.dt.int32
Alu = mybir.AluOpType
Act = mybir.ActivationFunctionType

# engine assignment knobs
POOL_FOLD2 = set()       # isolate: all fold2 on DVE
POOL_STT = set()  # scalar_tensor_tensor not supported on Pool engine

_CACHE = {}
LAST_RESULT = [None]


def _pix_map():
    """Column pixel mapping: (h, w) -> (partition, j)."""
    pix = np.empty((P, J), np.int64)
    hh, ww = np.meshgrid(np.arange(H), np.arange(W), indexing="ij")
    for h in range(H):
        for w in range(W):
            if w < 256:
                p, j = w % 128, 96 * (w // 128) + h
            else:
                p = (w - 256) + 64 * (h // 48)
                j = 192 + h % 48
            pix[p, j] = h * W + w
    return pix


PIX = _pix_map()


def _build():
    nc = bacc.Bacc("TRN2", target_bir_lowering=False, debug=False)

    xsum = nc.dram_tensor("xsum", [P, J * CPS], FP8, kind="ExternalInput")
    gdt = nc.dram_tensor("gdt", [P, KCAP * J], BF16, kind="ExternalInput")
    cmpd = nc.dram_tensor("cmpd", [P, KCAP], F32, kind="ExternalInput")
    SM = 4 + 4 + 1 + WE + H
    smalls = nc.dram_tensor("smalls", [N, SM], F32, kind="ExternalInput")
    outv = nc.dram_tensor("outv", [1, 1], F32, kind="ExternalOutput")

    SJMAX = max(STRIPJ)
    GH1 = 5 * J    # gather planes: slots 0-4 on sync ring
    GH2 = 12 * J   # slots 5-11 on gpsimd ring, 12-18 on sync ring

    with tile.TileContext(nc) as tc:
        with (
            tc.tile_pool(name="xs", bufs=1) as xsp,
            tc.tile_pool(name="es", bufs=2) as esp,
            tc.tile_pool(name="fold", bufs=2) as fp_,
            tc.tile_pool(name="map", bufs=1) as mapp,
            tc.tile_pool(name="sml", bufs=1) as smlp,
            tc.tile_pool(name="ps", bufs=2, space="PSUM") as psp,
        ):
            # ---- input DMAs ----
            # sync ring: tiny tensors first, then gather-plane head/tail
            sm_t = smlp.tile([N, SM], F32)
            nc.sync.dma_start(sm_t[:], smalls[:])
            cmp_t = smlp.tile([P, KCAP], F32)
            nc.sync.dma_start(cmp_t[:], cmpd[:])
            gd_t = xsp.tile([P, KCAP * J], BF16)
            nc.sync.dma_start(gd_t[:, 0:GH1], gdt[:, 0:GH1])
            # gpsimd ring: gather-plane mid, then late strips
            nc.gpsimd.dma_start(gd_t[:, GH1:GH2], gdt[:, GH1:GH2])
            nc.sync.dma_start(gd_t[:, GH2:], gdt[:, GH2:])
            # strips: 0-3 on scalar ring, 4-7 on gpsimd ring
            xst = xsp.tile([P, J * CPS], FP8)
            for s in range(NSTRIP):
                lo, hi = JOFF[s] * CPS, JOFF[s + 1] * CPS
                eng = nc.scalar if s < 4 else nc.gpsimd
                eng.dma_start(xst[:, lo:hi], xsum[:, lo:hi])

            # ---- ACT table warmup: load exp set while DMAs stream ----
            warm = smlp.tile([P, 1], F32)
            nc.vector.memset(warm[:], 0.0)
            nc.scalar.activation(warm[:], warm[:], Act.Exp)
            ones = smlp.tile([P, 1], F32)
            nc.vector.memset(ones[:], 1.0)

            box_t = sm_t[:, 0:4]
            sgn_t = sm_t[:, 4:8]
            pw2_t = sm_t[:, 8:9]
            iotw_t = sm_t[:, 9 : 9 + WE]
            ioth_t = sm_t[:, 9 + WE : 9 + WE + H]

            # ---- floor(u1,v1)/ceil(u2,v2): convert then fix up ----
            bxi = smlp.tile([N, 4], I32)
            nc.vector.tensor_copy(bxi[:], box_t)
            bxf = smlp.tile([N, 4], F32)
            nc.vector.tensor_copy(bxf[:], bxi[:])
            dlt = smlp.tile([N, 4], F32)
            nc.vector.tensor_tensor(dlt[:, 0:2], bxf[:, 0:2], sm_t[:, 0:2], Alu.is_gt)
            nc.vector.tensor_tensor(dlt[:, 2:4], bxf[:, 2:4], sm_t[:, 2:4], Alu.is_lt)
            nc.vector.tensor_tensor(dlt[:], dlt[:], sgn_t, Alu.mult)
            nc.vector.tensor_tensor(bxf[:], bxf[:], dlt[:], Alu.add)

            # ---- interval masks over the extended width iota ----
            mwa = smlp.tile([N, WE], F32)
            nc.vector.tensor_scalar(mwa[:], iotw_t, bxf[:, 0:1], None, Alu.is_ge)
            mw = smlp.tile([N, WE], F32)
            nc.vector.tensor_scalar(mw[:], iotw_t, bxf[:, 2:3], None, Alu.is_lt)
            nc.vector.tensor_tensor(mw[:], mw[:], mwa[:], Alu.mult)

            mha = smlp.tile([N, H], F32)
            nc.vector.tensor_scalar(mha[:], ioth_t, bxf[:, 1:2], None, Alu.is_ge)
            mhb = smlp.tile([N, H], F32)
            nc.vector.tensor_scalar(mhb[:], ioth_t, bxf[:, 3:4], None, Alu.is_lt)
            mhs = smlp.tile([N, H], F32)
            nc.vector.scalar_tensor_tensor(
                mhs[:], mha[:], pw2_t, mhb[:], Alu.mult, Alu.mult
            )

            # ---- raster into [128, 240] directly: v = sum 2^rank ----
            # stationary = width-chunk masks, moving = height masks
            psT = psp.tile([P, J], F32, tag="ps")
            nc.tensor.matmul(
                psT[:, 0:96], mw[:, 0:128], mhs[:], start=True, stop=True
            )
            nc.tensor.matmul(
                psT[:, 96:192], mw[:, 128:256], mhs[:], start=True, stop=True
            )
            # chunk 2 (w 256-319) split by h-half via zero-padded iotas
            nc.tensor.matmul(
                psT[:, 192:240], mw[:, 320:448], mhs[:, 0:48],
                start=True, stop=False,
            )
            nc.tensor.matmul(
                psT[:, 192:240], mw[:, 448:576], mhs[:, 48:96],
                start=False, stop=True,
            )

            vmap = mapp.tile([P, J], F32)
            ti = mapp.tile([P, J], I32)
            tb = mapp.tile([P, J], BF16)
            wt0 = mapp.tile([P, J], BF16)
            with tc.high_priority():
                nc.vector.tensor_copy(vmap[:], psT[:])
                # winner decode: t = 2^r* (clear mantissa), bf16 copy
                nc.vector.tensor_scalar(
                    ti[:], vmap[:].bitcast(I32), 0x7F800000, None, Alu.bitwise_and
                )
                nc.vector.tensor_copy(tb[:], ti[:].bitcast(F32))
                # weights: wt0 = 12 * covered (bf16; 12/13 exact)
                nc.vector.tensor_scalar(
                    wt0[:], vmap[:], 1.0, FG_W - BG_W, Alu.is_ge, Alu.mult
                )

            scr = mapp.tile([P, KCAP * J], BF16)
            sred = mapp.tile([P, J], F32)

            def gather_op(k):
                eng = nc.gpsimd if k in POOL_STT else nc.vector
                eng.scalar_tensor_tensor(
                    scr[:, k * J : (k + 1) * J],
                    tb[:],
                    cmp_t[:, k : k + 1],
                    gd_t[:, k * J : (k + 1) * J],
                    Alu.is_ge,
                    Alu.mult,
                )

            def strip_folds(s):
                js = STRIPJ[s]
                lo = JOFF[s] * CPS
                est = esp.tile([P, SJMAX * CPS], BF16, tag="est")
                ev = est[:, 0 : js * CPS]
                nc.scalar.activation(ev, xst[:, lo : lo + js * CPS], Act.Exp)
                e3 = ev.rearrange("p (j c) -> p j c", c=CPS)
                fA = fp_.tile([P, SJMAX * 42], BF16, tag="fA")
                fA3 = fA[:, 0 : js * 42].rearrange("p (j c) -> p j c", c=42)
                nc.vector.tensor_tensor(fA3, e3[:, :, 0:42], e3[:, :, 42:84], Alu.add)
                fB = fp_.tile([P, SJMAX * 21], BF16, tag="fB")
                fB3 = fB[:, 0 : js * 21].rearrange("p (j c) -> p j c", c=21)
                f2eng = nc.gpsimd if s in POOL_FOLD2 else nc.vector
                f2eng.tensor_tensor(fB3, fA3[:, :, 0:21], fA3[:, :, 21:42], Alu.add)
                nc.vector.tensor_reduce(
                    sred[:, JOFF[s] : JOFF[s + 1]],
                    fB3,
                    axis=mybir.AxisListType.X,
                    op=Alu.add,
                )

            # DVE program order tuned to expected data readiness:
            for k in range(0, 5):
                gather_op(k)
            strip_folds(0)
            strip_folds(1)
            for k in range(5, 12):
                gather_op(k)
            strip_folds(2)
            for k in range(12, 15):
                gather_op(k)
            for k in sorted(POOL_STT):
                gather_op(k)
            strip_folds(3)

            # ---- gather tree: 20 = (8+8) + (2+2) ----
            t8 = mapp.tile([P, 8 * J], BF16)
            nc.vector.tensor_tensor(
                t8[:], scr[:, 0 : 8 * J], scr[:, 8 * J : 16 * J], Alu.add
            )
            strip_folds(4)
            e4 = mapp.tile([P, 2 * J], BF16)
            nc.vector.tensor_tensor(
                e4[:], scr[:, 16 * J : 18 * J], scr[:, 18 * J : 20 * J], Alu.add
            )
            t4 = mapp.tile([P, 4 * J], BF16)
            nc.vector.tensor_tensor(
                t4[:], t8[:, 0 : 4 * J], t8[:, 4 * J : 8 * J], Alu.add
            )
            strip_folds(5)
            t2 = mapp.tile([P, 2 * J], BF16)
            nc.vector.tensor_tensor(
                t2[:], t4[:, 0 : 2 * J], t4[:, 2 * J : 4 * J], Alu.add
            )
            e2 = mapp.tile([P, J], BF16)
            nc.vector.tensor_tensor(e2[:], e4[:, 0:J], e4[:, J : 2 * J], Alu.add)
            t1 = mapp.tile([P, J], BF16)
            nc.vector.tensor_tensor(t1[:], t2[:, 0:J], t2[:, J : 2 * J], Alu.add)
            gat = mapp.tile([P, J], BF16)
            nc.vector.tensor_tensor(gat[:], t1[:], e2[:], Alu.add)
            strip_folds(6)
            strip_folds(7)

            # ---- lse = ln(S) via exponent bit trick (bf16 out) ----
            lse = mapp.tile([P, J], BF16)
            nc.vector.tensor_scalar(
                lse[:],
                sred[:].bitcast(I32),
                LN2 / (1 << 23),
                -(127.0 - SIG) * LN2,
                Alu.mult,
                Alu.add,
            )

            # ---- focal epilogue (bf16) ----
            logp = mapp.tile([P, J], BF16)
            nc.vector.tensor_tensor(logp[:], gat[:], lse[:], Alu.subtract)
            pt = mapp.tile([P, J], BF16)
            nc.scalar.activation(pt[:], logp[:], Act.Exp)
            um = mapp.tile([P, J], BF16)
            nc.scalar.activation(um[:], pt[:], Act.Identity, scale=-1.0, bias=1.0)
            tmp = mapp.tile([P, J], BF16)
            nc.vector.scalar_tensor_tensor(
                tmp[:], um[:], -ALPHA, um[:], Alu.mult, Alu.mult
            )
            wl = mapp.tile([P, J], BF16)
            nc.vector.scalar_tensor_tensor(
                wl[:], wt0[:], 1.0, logp[:], Alu.add, Alu.mult
            )
            junk = mapp.tile([P, J], BF16)
            nc.vector.tensor_tensor(junk[:], tmp[:], wl[:], Alu.mult)
            acc = mapp.tile([P, 1], F32)
            nc.vector.tensor_reduce(
                acc[:], junk[:], axis=mybir.AxisListType.X, op=Alu.add
            )
            # partition reduce on the idle PE: tot = ones^T @ acc
            pacc = psp.tile([1, 1], F32, tag="pacc")
            nc.tensor.matmul(pacc[:], acc[:], ones[:], start=True, stop=True)
            tot = mapp.tile([1, 1], F32)
            nc.vector.tensor_copy(tot[:], pacc[:])
            nc.sync.dma_start(outv[:], tot[0:1, 0:1])

    nc.finalize()
    return nc


def _ref_bin(d):
    """Per-box target bin, replicating the reference's float32 LID binning."""
    d = np.float32(d)
    a = np.float32(1.0) + np.float32(8.0) * (d - np.float32(DEPTH_MIN)) / np.float32(
        BIN_SIZE
    )
    idx = np.float32(-0.5) + np.float32(0.5) * np.sqrt(a, dtype=np.float32)
    return int(np.int32(idx))


def _host_prep(depth_logits, gt_boxes2d, gt_center_depth):
    xt = np.transpose(depth_logits, (0, 2, 3, 1)).reshape(B, HW, C)
    boxes = gt_boxes2d.reshape(B, N, 4)
    depths = gt_center_depth.reshape(B, N)

    fbox = np.concatenate(
        [np.floor(boxes[:, :, :2]), np.ceil(boxes[:, :, 2:])], axis=2
    )

    SM = 4 + 4 + 1 + WE + H
    xsum = np.empty((B, P, J * CPS), ml_dtypes.float8_e4m3fn)
    gdt = np.zeros((B, P, KCAP * J), ml_dtypes.bfloat16)
    cmpv = np.full((B, P, KCAP), CMP_PAD, np.float32)
    smalls = np.empty((B, N, SM), np.float32)

    # extended width iota: [0..319, za(128), zb(128)]
    iotw = np.full(WE, IOTA_DEAD, np.float32)
    iotw[0:W] = np.arange(W)
    # za chunk (mask cols 320:448): rows 0-63 live -> w 256-319, h<48
    iotw[W : W + 64] = np.arange(256, 320)
    # zb chunk (mask cols 448:576): rows 64-127 live -> w 256-319, h>=48
    iotw[W + 192 : W + 256] = np.arange(256, 320)

    for b in range(B):
        # rank: farthest depth = rank 0, nearest = rank N-1
        order = np.argsort(-depths[b], kind="stable")
        smalls[b, :, 0:4] = boxes[b][order]
        smalls[b, :, 4:8] = np.array([-1.0, -1.0, 1.0, 1.0], np.float32)
        smalls[b, :, 8] = (2.0 ** np.arange(N)).astype(np.float32)
        smalls[b, :, 9 : 9 + WE] = iotw
        smalls[b, :, 9 + WE : 9 + WE + H] = np.arange(H, dtype=np.float32)

        fb = fbox[b][order]
        bins = np.array([_ref_bin(depths[b][o]) for o in order], np.int32)
        u1 = fb[:, 0].astype(int)
        v1 = fb[:, 1].astype(int)
        u2 = fb[:, 2].astype(int)
        v2 = fb[:, 3].astype(int)

        # f32-exactness guard for the power-sum raster: counts per pixel
        cnt = np.zeros((H, W), np.int32)
        for n in range(N):
            cnt[max(v1[n], 0) : v2[n], max(u1[n], 0) : u2[n]] += 1
        assert cnt.max() <= 23, "overlap too deep for exact f32 power-sum"

        # sum region: 84 channels, pixel-major [P, j, c] (column mapping)
        xb = np.full((HW, CPS), PAD_LOGIT, np.float32)
        xb[:, :C] = xt[b]
        xsum[b] = xb[PIX.reshape(-1)].reshape(P, J * CPS).astype(
            ml_dtypes.float8_e4m3fn
        )

        # per-partition Abel slots
        xpix = xt[b][PIX.reshape(-1)].reshape(P, J, C)
        for p in range(P):
            cols = [(p, 0, 96), (128 + p, 0, 96)]
            if p < 64:
                cols.append((256 + p, 0, 48))
            else:
                cols.append((256 + p - 64, 48, 96))
            rel = [
                n
                for n in range(N)
                if any(
                    u1[n] <= cw < u2[n] and not (v2[n] <= ha or v1[n] >= hb)
                    for (cw, ha, hb) in cols
                )
            ]
            # merged-bin runs over ascending rank (bins non-increasing)
            runs = []  # (rlo, bin)
            for n in rel:
                if not runs or bins[n] != runs[-1][1]:
                    runs.append((n, bins[n]))
            while len(runs) > KCAP - 1:
                # merge the run whose bin is closest to its predecessor's
                dd = [abs(runs[i][1] - runs[i - 1][1]) for i in range(1, len(runs))]
                i = 1 + int(np.argmin(dd))
                del runs[i]
            prev = xpix[p, :, NUM_BINS].astype(np.float32)
            cmpv[b, p, 0] = 0.0
            gdt[b, p, 0:J] = prev
            for k, (rlo, bn) in enumerate(runs, start=1):
                cur = xpix[p, :, bn].astype(np.float32)
                cmpv[b, p, k] = float(2.0**rlo)
                gdt[b, p, k * J : (k + 1) * J] = cur - prev
                prev = cur

    return xsum, gdt, cmpv, smalls


def kernel(depth_logits, gt_boxes2d, gt_boxes3d, gt_center_depth, num_gt_per_img):
    depth_logits = np.asarray(depth_logits, dtype=np.float32)
    gt_boxes2d = np.asarray(gt_boxes2d, dtype=np.float32)
    gt_center_depth = np.asarray(gt_center_depth, dtype=np.float32)
    assert int(num_gt_per_img) == N

    xsum, gdt, cmpv, smalls = _host_prep(depth_logits, gt_boxes2d, gt_center_depth)

    if "nc" not in _CACHE:
        _CACHE["nc"] = _build()
    nc = _CACHE["nc"]

    in_maps = []
    for b in range(B):
        in_maps.append(
            {
                "xsum": np.ascontiguousarray(xsum[b]),
                "gdt": np.ascontiguousarray(gdt[b]),
                "cmpd": np.ascontiguousarray(cmpv[b]),
                "smalls": np.ascontiguousarray(smalls[b]),
            }
        )

    res = run_bass_kernel_spmd(nc, in_maps, core_ids=list(range(B)))
    LAST_RESULT[0] = res
    total = 0.0
    for b in range(B):
        total += float(res.results[b]["outv"][0, 0])
    return np.float32(total / (B * H * W))
